# revision 12
# baseline (speedup 1.0000x reference)
"""Trainium2 Bass kernel for nn_CustomEmbed (ConvNeXt-style embed stack).

Data-parallel over batch: 8 images per NeuronCore x 8 cores.
Channel halves are split 96/96 everywhere so region convs can run as
fp8e4m3 DoubleRow matmuls (2 K-planes of 96 channels per instruction,
0.5 cycles/row): conv stays bf16 where precision matters (conv1/2/3,
residual path), regions are fp8 with full-precision residuals.

Region conv layout: each 7x7 tile image is stored flat-padded as
[10 rows x 8 cols] fp8 (rows 0/8 and col 0 zero), so every 3x3 tap is
one contiguous 56-element window: rhs AP = [96, 2planes, IMG, 56],
out = [96, IMG, 7, 8] PSUM with col 7 junk. conv1->region1 is fused
per group (no h1 DRAM roundtrip); all intermediates live in SBUF.
"""
import numpy as np
import ml_dtypes

import concourse.tile as tile
from concourse import bacc, mybir
from concourse.bass_utils import run_bass_kernel_spmd

AF = mybir.ActivationFunctionType
PM = mybir.MatmulPerfMode
dt = mybir.dt
BF16 = ml_dtypes.bfloat16
F8 = ml_dtypes.float8_e4m3

EPS = 1e-5
B = 64
NCORE = 8
IMG = B // NCORE          # 8 images per core
C4 = 192
ED = 768
H = 96                    # channel half
BLK = 80                  # padded flat block per image: 10 rows x 8 cols

TAPS = [(dy, dx) for dy in (-1, 0, 1) for dx in (-1, 0, 1)]
PHASES = [(0, 0), (0, 1), (1, 0), (1, 1)]          # p = 2*sy + sx


# ---------------------------------------------------------------- host prep

def _fold(w, g, b, m, v):
    """Fold inference BN into conv weight (scaled along axis 0) + bias."""
    inv = (g / np.sqrt(v + EPS)).astype(np.float32)
    bias = (b - m * inv).astype(np.float32)
    return w.astype(np.float32) * inv.reshape(-1, 1, 1, 1), bias


def prep_weights(inp):
    """Returns dict of device-ready weight arrays (shared across cores)."""
    out = {}
    w1, b1 = _fold(inp["conv1_w"], inp["bn1_g"], inp["bn1_b"], inp["bn1_m"],
                   inp["bn1_v"])                       # (192, 3, 4, 4)
    out["w1"] = np.ascontiguousarray(
        w1.transpose(1, 2, 3, 0).reshape(48, C4)).astype(BF16)
    out["b1"] = b1.reshape(C4, 1)

    # region weights -> fp8 [G, 96, tap, plane, co]
    for nm, G in (("r1", 64), ("r2", 16)):
        w = np.asarray(inp[f"{nm}_w"], np.float32)      # (G, co, ci, 3, 3)
        inv = (np.asarray(inp[f"{nm}_g"], np.float32) /
               np.sqrt(np.asarray(inp[f"{nm}_v"], np.float32) + EPS))
        bias = (np.asarray(inp[f"{nm}_b"], np.float32) -
                np.asarray(inp[f"{nm}_m"], np.float32) * inv)
        w = w * inv[:, :, None, None, None]
        wt = w.transpose(0, 2, 3, 4, 1).reshape(G, 2, H, 9, C4)
        wt = np.ascontiguousarray(wt.transpose(0, 2, 3, 1, 4))
        out[f"{nm}w"] = np.clip(wt, -240.0, 240.0).astype(F8)
        out[f"{nm}b"] = np.ascontiguousarray(bias.T)    # (192, G)

    w2, b2 = _fold(inp["conv2_w"], inp["bn2_g"], inp["bn2_b"], inp["bn2_m"],
                   inp["bn2_v"])                       # (192, 192, 2, 2)
    out["w2"] = np.ascontiguousarray(
        w2.transpose(1, 2, 3, 0).reshape(C4, 4, C4)).astype(BF16)
    out["b2"] = b2.reshape(C4, 1)

    w3, b3 = _fold(inp["conv3_w"], inp["bn3_g"], inp["bn3_b"], inp["bn3_m"],
                   inp["bn3_v"])                       # (768, 192, 2, 2)
    out["w3"] = np.ascontiguousarray(
        w3.transpose(1, 2, 3, 0).reshape(C4, 4, ED)).astype(BF16)
    out["b3"] = np.ascontiguousarray(b3.reshape(6, 128).T)   # (128, 6)
    return out


def prep_a1(x_core):
    """x (IMG,3,224,224) fp32 -> a1 [48, 64 groups, IMG*49] bf16 im2col."""
    i = x_core.shape[0]
    t = x_core.reshape(i, 3, 8, 7, 4, 8, 7, 4)     # (i, c, gy, y, dy, gx, x, dx)
    t = t.transpose(1, 4, 7, 2, 5, 0, 3, 6)        # (c, dy, dx, gy, gx, i, y, x)
    return np.ascontiguousarray(t.reshape(48, 64, i * 49)).astype(BF16)


# ------------------------------------------------------------- device build

def build_program():
    nc = bacc.Bacc("TRN2", target_bir_lowering=False)

    a1_d = nc.declare_dram_parameter("a1", [48, 64, IMG * 49], dt.bfloat16, isOutput=False)
    w1_d = nc.declare_dram_parameter("w1", [48, C4], dt.bfloat16, isOutput=False)
    b1_d = nc.declare_dram_parameter("b1", [C4, 1], dt.float32, isOutput=False)
    r1w_d = nc.declare_dram_parameter("r1w", [64, H, 9, 2, C4], dt.float8e4, isOutput=False)
    r1b_d = nc.declare_dram_parameter("r1b", [C4, 64], dt.float32, isOutput=False)
    w2_d = nc.declare_dram_parameter("w2", [C4, 4, C4], dt.bfloat16, isOutput=False)
    b2_d = nc.declare_dram_parameter("b2", [C4, 1], dt.float32, isOutput=False)
    r2w_d = nc.declare_dram_parameter("r2w", [16, H, 9, 2, C4], dt.float8e4, isOutput=False)
    r2b_d = nc.declare_dram_parameter("r2b", [C4, 16], dt.float32, isOutput=False)
    w3_d = nc.declare_dram_parameter("w3", [C4, 4, ED], dt.bfloat16, isOutput=False)
    b3_d = nc.declare_dram_parameter("b3", [128, 6], dt.float32, isOutput=False)
    out_d = nc.declare_dram_parameter("out3", [ED, IMG * 196], dt.float32,
                                      isOutput=True)

    with tile.TileContext(nc) as tc:
        with (
            tc.tile_pool(name="pers", bufs=1) as pp,
            tc.tile_pool(name="const", bufs=1) as cp,
            tc.tile_pool(name="wpool", bufs=4) as wp,
            tc.tile_pool(name="t8p", bufs=4) as tp,
            tc.tile_pool(name="io", bufs=3) as io,
            tc.tile_pool(name="a1p", bufs=4) as ap,
            tc.tile_pool(name="ps", bufs=4, space="PSUM") as ps,
        ):
            # persistent SBUF intermediates (channel halves A=0:96, B=96:192)
            h1bA = pp.tile([H, IMG, 56, 56], dt.bfloat16, name="h1bA")
            h1bB = pp.tile([H, IMG, 56, 56], dt.bfloat16, name="h1bB")
            h2A = pp.tile([H, IMG, 28, 28], dt.bfloat16, name="h2A")
            h2B = pp.tile([H, IMG, 28, 28], dt.bfloat16, name="h2B")
            h2bA = pp.tile([H, IMG, 28, 28], dt.bfloat16, name="h2bA")
            h2bB = pp.tile([H, IMG, 28, 28], dt.bfloat16, name="h2bB")

            # ---- resident constants
            w1t = cp.tile([48, C4], dt.bfloat16, name="w1t")
            nc.sync.dma_start(w1t[:], w1_d[:])
            b1A = cp.tile([H, 1], dt.float32, name="b1A")
            nc.sync.dma_start(b1A[:], b1_d[0:H])
            b1B = cp.tile([H, 1], dt.float32, name="b1B")
            nc.sync.dma_start(b1B[:], b1_d[H:C4])
            r1bA = cp.tile([H, 64], dt.float32, name="r1bA")
            nc.sync.dma_start(r1bA[:], r1b_d[0:H])
            r1bB = cp.tile([H, 64], dt.float32, name="r1bB")
            nc.sync.dma_start(r1bB[:], r1b_d[H:C4])
            w2a = cp.tile([H, 4, C4], dt.bfloat16, name="w2a")
            nc.scalar.dma_start(w2a[:], w2_d[0:H])
            w2b = cp.tile([H, 4, C4], dt.bfloat16, name="w2b")
            nc.scalar.dma_start(w2b[:], w2_d[H:C4])
            b2A = cp.tile([H, 1], dt.float32, name="b2A")
            nc.scalar.dma_start(b2A[:], b2_d[0:H])
            b2B = cp.tile([H, 1], dt.float32, name="b2B")
            nc.scalar.dma_start(b2B[:], b2_d[H:C4])
            r2bA = cp.tile([H, 16], dt.float32, name="r2bA")
            nc.scalar.dma_start(r2bA[:], r2b_d[0:H])
            r2bB = cp.tile([H, 16], dt.float32, name="r2bB")
            nc.scalar.dma_start(r2bB[:], r2b_d[H:C4])
            w3a = cp.tile([H, 4, ED], dt.bfloat16, name="w3a")
            nc.scalar.dma_start(w3a[:], w3_d[0:H])
            w3b = cp.tile([H, 4, ED], dt.bfloat16, name="w3b")
            nc.scalar.dma_start(w3b[:], w3_d[H:C4])
            b3t = cp.tile([128, 6], dt.float32, name="b3t")
            nc.scalar.dma_start(b3t[:], b3_d[:])

            # ---- fused conv1 + region1, software-pipelined over 64 groups
            def conv1_stage(g):
                a1t = ap.tile([48, IMG * 49], dt.bfloat16, name="a1t")
                nc.sync.dma_start(a1t[:], a1_d[:, g, :])
                w8 = wp.tile([H, 9, 2, C4], dt.float8e4, name="w8")
                nc.gpsimd.dma_start(w8[:], r1w_d[g])
                psA0 = ps.tile([H, IMG, 7, 7], dt.float32, name="psA0",
                               tag="pm0")
                nc.tensor.matmul(psA0[:], w1t[:, 0:H], a1t[:],
                                 start=True, stop=True)
                psA1 = ps.tile([H, IMG, 7, 7], dt.float32, name="psA1",
                               tag="pm1")
                nc.tensor.matmul(psA1[:], w1t[:, H:C4], a1t[:],
                                 start=True, stop=True)
                tA = io.tile([H, IMG, 7, 7], dt.bfloat16, name="tA")
                nc.scalar.activation(tA[:], psA0[:], AF.Gelu, bias=b1A[:])
                tB = io.tile([H, IMG, 7, 7], dt.bfloat16, name="tB")
                nc.scalar.activation(tB[:], psA1[:], AF.Gelu, bias=b1B[:])
                t8 = tp.tile([H, 2, IMG, 10, 8], dt.float8e4, name="t8")
                if g < 4:      # zero pads once per physical pool buffer
                    nc.vector.memset(
                        t8[:].rearrange("p a i r c -> p (a i r c)"), 0.0)
                nc.vector.tensor_copy(t8[:, 0, :, 1:8, 1:8], tA[:])
                nc.gpsimd.tensor_copy(t8[:, 1, :, 1:8, 1:8], tB[:])
                return tA, tB, t8, w8

            def region_stage(g, tA, tB, t8, w8, biasA, biasB, grid, dstA,
                             dstB):
                gy, gx = divmod(g, grid)
                flat = t8[:].rearrange("p a i r c -> p a i (r c)")
                for mi, (bias, dst, tt) in enumerate(
                        ((biasA, dstA, tA), (biasB, dstB, tB))):
                    psB = ps.tile([H, IMG, 7, 8], dt.float32,
                                  name=f"psB{mi}", tag=f"pm{mi}")
                    pso = psB[:].rearrange("p i r c -> p i (r c)")
                    for it, (dy, dx) in enumerate(TAPS):
                        o = 9 + 8 * dy + dx
                        nc.tensor.matmul(
                            pso, w8[:, it, :, H * mi:H * mi + H],
                            flat[:, :, :, o:o + 56],
                            start=(it == 0), stop=(it == 8),
                            perf_mode=PM.DoubleRow)
                    gB = io.tile([H, IMG, 7, 7], dt.bfloat16, name=f"gB{mi}")
                    nc.scalar.activation(gB[:], psB[:, :, :, 0:7], AF.Gelu,
                                         bias=bias[:, g:g + 1])
                    nc.vector.tensor_add(
                        dst[:, :, 7 * gy:7 * gy + 7, 7 * gx:7 * gx + 7],
                        gB[:], tt[:])

            pipe = {}
            for g in range(66):
                if g < 64:
                    pipe[g] = conv1_stage(g)
                if g >= 2:
                    region_stage(g - 2, *pipe.pop(g - 2), r1bA, r1bB, 8,
                                 h1bA, h1bB)

            # ---- conv2 + BN + GELU -> h2A/h2B (SBUF)
            for i in range(IMG):
                for hh in range(2):
                    for mi in range(2):
                        psC = ps.tile([H, 14, 28], dt.float32, name=f"psC{mi}",
                                      tag=f"pm{mi}")
                        imm = 0
                        for p, (sy, sx) in enumerate(PHASES):
                            r0 = sy + 28 * hh
                            rend = 28 * hh + 28
                            nc.tensor.matmul(
                                psC[:], w2a[:, p, H * mi:H * mi + H],
                                h1bA[:, i, r0:rend:2, sx::2],
                                start=(imm == 0), stop=False)
                            imm += 1
                            nc.tensor.matmul(
                                psC[:], w2b[:, p, H * mi:H * mi + H],
                                h1bB[:, i, r0:rend:2, sx::2],
                                start=False, stop=(imm == 7))
                            imm += 1
                        dst = (h2A, h2B)[mi]
                        bias = (b2A, b2B)[mi]
                        nc.scalar.activation(dst[:, i, 14 * hh:14 * hh + 14, :],
                                             psC[:], AF.Gelu, bias=bias[:])

            # ---- region2 (16 groups), software-pipelined
            def prep2(g):
                gy, gx = divmod(g, 4)
                y0, x0 = 7 * gy, 7 * gx
                w8 = wp.tile([H, 9, 2, C4], dt.float8e4, name="w8")
                nc.gpsimd.dma_start(w8[:], r2w_d[g])
                t8 = tp.tile([H, 2, IMG, 10, 8], dt.float8e4, name="t8")
                tAv = h2A[:, :, y0:y0 + 7, x0:x0 + 7]
                tBv = h2B[:, :, y0:y0 + 7, x0:x0 + 7]
                nc.vector.tensor_copy(t8[:, 0, :, 1:8, 1:8], tAv)
                nc.gpsimd.tensor_copy(t8[:, 1, :, 1:8, 1:8], tBv)
                return tAv, tBv, t8, w8

            pipe2 = {}
            for g in range(17):
                if g < 16:
                    pipe2[g] = prep2(g)
                if g >= 1:
                    region_stage(g - 1, *pipe2.pop(g - 1), r2bA, r2bB, 4,
                                 h2bA, h2bB)

            # ---- conv3 + BN -> out3 (DRAM)
            for ip in range(4):
                for j in range(6):
                    psE = ps.tile([128, 2, 14, 14], dt.float32, name="psE",
                                  tag=f"pm{(ip * 6 + j) % 2}")
                    imm = 0
                    for p, (sy, sx) in enumerate(PHASES):
                        nc.tensor.matmul(
                            psE[:], w3a[:, p, 128 * j:128 * j + 128],
                            h2bA[:, 2 * ip:2 * ip + 2, sy::2, sx::2],
                            start=(imm == 0), stop=False)
                        imm += 1
                        nc.tensor.matmul(
                            psE[:], w3b[:, p, 128 * j:128 * j + 128],
                            h2bB[:, 2 * ip:2 * ip + 2, sy::2, sx::2],
                            start=False, stop=(imm == 7))
                        imm += 1
                    oE = io.tile([128, 2, 14, 14], dt.float32, name="oE")
                    nc.vector.tensor_scalar_add(oE[:], psE[:], b3t[:, j:j + 1])
                    nc.sync.dma_start(
                        out_d[128 * j:128 * j + 128,
                              392 * ip:392 * (ip + 1)],
                        oE[:])

    nc.compile()
    return nc


_NC_CACHE = {}


def _get_program():
    if "nc" not in _NC_CACHE:
        _NC_CACHE["nc"] = build_program()
    return _NC_CACHE["nc"]


def run(inputs, trace=False):
    """Returns (output, BassKernelResults)."""
    nc = _get_program()
    wts = prep_weights(inputs)
    x = np.asarray(inputs["x"], np.float32)
    in_maps = []
    for c in range(NCORE):
        m = dict(wts)
        m["a1"] = prep_a1(x[c * IMG:(c + 1) * IMG])
        in_maps.append(m)
    res = run_bass_kernel_spmd(nc, in_maps, list(range(NCORE)), trace=trace)
    # gather: per-core (768, IMG*196) -> (B, 196, 768)
    outs = [np.asarray(r["out3"]).reshape(ED, IMG, 196).transpose(1, 2, 0)
            for r in res.results]
    full = np.ascontiguousarray(np.concatenate(outs, axis=0), dtype=np.float32)
    return full, res


def kernel(**inputs):
    return run(inputs)[0]
